# revision 5
# baseline (speedup 1.0000x reference)
"""Trainium2 Bass kernel for CustomWindowMHA (sparse window+dilated attention).

Problem (hardcoded):
  x: (2, 2048, 1024) f32, qkv: (3072, 1024) f32, wo: (1024, 1024) f32
  H=16 heads, dh=64, window=128, dilation=4.
  out = softmax(mask(QK^T/8)) V @ wo^T          (B, S, D) f32

Sharding: 16 heads / 8 cores = 2 heads per core (head-parallel).  Each core
computes its 2 heads' attention output O_d (n=4096, 128) and the partial
Y_d = O_d @ wo[:, d-slice]^T (4096, 1024).  Host sums the 8 partials.

Per-core layout (activations chained transposed; fp32 PSUM accumulation):
  Q/K projections run as fp8-e4m3 DoubleRow matmuls (x and wqk scaled by 16,
  two f-tiles of the K=1024 contraction packed per instruction => 4x fewer
  PE cycles); the 1/(16*16*8) score descale is folded into the exp scale.
  V projection stays bf16 (its error hits the output directly).
  Per j-block, both heads' scores land in one two-bank (128, 1024) PSUM
  tile so a single scaled Exp covers them; the 0/1 W01 band mask is applied
  multiplicatively on DVE.  AV accumulates per head into (65, 512) PSUM with
  a ones-column carrying the softmax denominator.  A deep-dilated residue-
  compacted stream (all-valid, maskless) shares the same accumulators.
  Normalization: one reciprocal pair -> single gpsimd partition_broadcast ->
  two DVE muls writing both heads into ONE stacked (128, 512) bf16 tile (the
  second write is partition-shifted), so the output projection runs as plain
  K=128 matmuls with no split/DMA shuffle.  Y PSUM->SBUF copies are spread
  across ACT/DVE/Pool to balance engine load.
"""

import os
import sys

import numpy as np

for _p in ("/opt/trn_rl_repo",):
    if _p not in sys.path and os.path.isdir(_p):
        sys.path.append(_p)

import ml_dtypes
import concourse.bacc as bacc
import concourse.bass as bass
import concourse.mybir as mybir
import concourse.tile as tile
from concourse.bass_utils import run_bass_kernel_spmd
from concourse.masks import make_identity

F32 = mybir.dt.float32
BF16 = mybir.dt.bfloat16
FP8 = mybir.dt.float8e4
NPBF16 = ml_dtypes.bfloat16
NPFP8 = ml_dtypes.float8_e4m3
DR = mybir.MatmulPerfMode.DoubleRow

B, S, D = 2, 2048, 1024
H, DH = 16, 64
WINDOW, DIL = 128, 4
NCORES = 8
N = B * S          # 4096 rows total
DH2 = 2 * DH       # 128 dims per core (2 heads)
NCH = N // 512     # 8 n-chunks of 512
CPB = S // 512     # 4 i-chunks per batch
JB = S // 128      # 16 j-blocks per batch
T0 = 384           # W01 band offset: delta = t - T0 - j'
TW = T0 + 512 * (CPB - 1) + 512  # 2432 band width
WSCALE = 16.0                    # fp8 Q/K weight scale
EXP_SCALE = 1.0 / (WSCALE * WSCALE * 8.0)  # descale folded into exp


def build_tile_kernel(tc):
    nc = tc.nc

    xT = nc.dram_tensor("xT", [128, 8, N], BF16, kind="ExternalInput").ap()
    xT8 = nc.dram_tensor("xT8", [128, 8, N], FP8, kind="ExternalInput").ap()
    wqk8 = nc.dram_tensor("wqk8", [128, 8, 2 * DH2], FP8, kind="ExternalInput").ap()
    wv16 = nc.dram_tensor("wv16", [128, 8, DH2], BF16, kind="ExternalInput").ap()
    woT = nc.dram_tensor("woT", [DH2, D], BF16, kind="ExternalInput").ap()
    w01 = nc.dram_tensor("w01", [128, TW], BF16, kind="ExternalInput").ap()
    y = nc.dram_tensor("y", [N, D], BF16, kind="ExternalOutput").ap()

    with (
        tc.tile_pool(name="const", bufs=1) as cpool,
        tc.tile_pool(name="xin", bufs=3) as xpool,
        tc.tile_pool(name="x8in", bufs=3) as x8pool,
        tc.tile_pool(name="qk", bufs=8) as qkpool,
        tc.tile_pool(name="vtp", bufs=2) as vtpool,
        tc.tile_pool(name="pers", bufs=1) as perspool,
        tc.tile_pool(name="exp", bufs=4) as epool,
        tc.tile_pool(name="ot2", bufs=4) as opool,
        tc.tile_pool(name="small", bufs=4) as spool,
        tc.tile_pool(name="yst", bufs=2) as ypool,
    ):
        # ---- constants / weights in SBUF ----
        wqk_sb = cpool.tile([128, 8, 2 * DH2], FP8, name="wqk_sb")
        nc.sync.dma_start(wqk_sb, wqk8)
        wv_sb = cpool.tile([128, 8, DH2], BF16, name="wv_sb")
        nc.sync.dma_start(wv_sb, wv16)
        wo_sb = cpool.tile([128, D], BF16, name="wo_sb")
        w01_sb = cpool.tile([128, TW], BF16, name="w01_sb")
        ident = cpool.tile([128, 128], BF16, name="ident")
        make_identity(nc, ident)

        # per-head V with ones column, interleaved: per j-block g a 130-col
        # group [V_h0(64) | 1 | V_h1(64) | 1]
        vhh = perspool.tile([128, 2 * JB * 130], BF16, name="vhh")
        nc.vector.memset(
            vhh.rearrange("p (g w) -> p g w", w=65)[:, :, 64:65], 1.0)
        # residue-gathered V for the compact stream: (b, r, jmb<3) 130-groups
        vrr = perspool.tile([128, B * 4 * 3 * 130], BF16, name="vrr")
        nc.vector.memset(
            vrr.rearrange("p (g w) -> p g w", w=65)[:, :, 64:65], 1.0)

        qts = [None] * NCH
        ktb = [None] * B
        vtb = [None] * B

        for bb in range(B):
            with (
                tc.tile_pool(name=f"pp1_{bb}", bufs=2, space="PSUM") as pp1,
                tc.tile_pool(name=f"psc_{bb}", bufs=2, space="PSUM") as psc,
                tc.tile_pool(name=f"pot_{bb}", bufs=2, space="PSUM") as pot,
            ):
                b = bb
                kt = qkpool.tile([128, S], BF16, tag="kt", name=f"ktb{bb}", bufs=2)
                vt = vtpool.tile([128, S], BF16, tag="vt", name=f"vtb{bb}", bufs=2)
                ktb[bb] = kt
                vtb[bb] = vt
                vt4 = vt.rearrange("p (j r) -> p r j", r=4)
                def do_attn(c, qci):
                    # ---------- attention + output proj for chunk cc ----------
                    ot0 = pot.tile([65, 512], F32, tag="ot", name=f"ot0_{qci}")
                    ot1 = pot.tile([65, 512], F32, tag="ot", name=f"ot1_{qci}")
                    ots = (ot0, ot1)
                    ot4 = [o.rearrange("p (i r) -> p r i", r=4) for o in ots]
                    jcap = 128 * (4 * c - 1)
                    jb_lo = max(0, 4 * c - 1)
                    qt4 = [qts[qci][h * 64:(h + 1) * 64, :].rearrange(
                        "p (i r) -> p r i", r=4) for h in (0, 1)]
                    kt4 = [ktb[b][h * 64:(h + 1) * 64, :].rearrange(
                        "p (j r) -> p r j", r=4) for h in (0, 1)]

                    # ---- window / mixed stream ----
                    for jb in range(jb_lo, 4 * c + 4):
                        g = b * JB + jb
                        im0 = max(0, 128 * jb - 512 * c)
                        sc2 = psc.tile([128, 1024], F32, tag="sc",
                                       name=f"sc{qci}_{jb}")
                        sc2h = sc2.rearrange("p (h i) -> p h i", h=2)
                        for h in (0, 1):
                            nc.tensor.matmul(
                                sc2h[:, h, im0:512],
                                ktb[b][h * 64:(h + 1) * 64, jb * 128:(jb + 1) * 128],
                                qts[qci][h * 64:(h + 1) * 64, im0:512],
                                start=True, stop=True, skip_group_check=True)
                        e2 = epool.tile([128, 2, 512], BF16, tag="e", name=f"e{qci}_{jb}")
                        nc.scalar.activation(e2[:, :, im0:512], sc2h[:, :, im0:512],
                                             mybir.ActivationFunctionType.Exp,
                                             scale=EXP_SCALE)
                        em2 = epool.tile([128, 2, 512], BF16, tag="em", name=f"em{qci}_{jb}")
                        t0 = T0 + 512 * c - 128 * jb
                        wsl = w01_sb[:, t0 + im0:t0 + 512]
                        wb = bass.AP(tensor=wsl.tensor, offset=wsl.offset,
                                     ap=[list(wsl.ap[0]), [0, 2], list(wsl.ap[1])])
                        nc.vector.tensor_mul(em2[:, :, im0:512], e2[:, :, im0:512], wb)
                        for h in (0, 1):
                            nc.tensor.matmul(
                                ots[h][:, im0:512],
                                vhh[:, g * 130 + h * 65:g * 130 + h * 65 + 65],
                                em2[:, h, im0:512],
                                start=(jb == jb_lo), stop=False, skip_group_check=True)

                    # ---- deep-dilated compact stream (all-valid; maskless) ----
                    jmtot = max(jcap, 0) // 4
                    njmb = (jmtot + 127) // 128
                    for jmb in range(njmb):
                        jm0 = 128 * jmb
                        jmw = min(128, jmtot - jm0)
                        scd2 = psc.tile([128, 1024], F32, tag="sc",
                                        name=f"scd{qci}_{jmb}")
                        scd2h = scd2.rearrange("p (h i) -> p h i", h=2)
                        for r in range(4):
                            for h in (0, 1):
                                nc.tensor.matmul(
                                    scd2h[0:jmw, h, r * 128:(r + 1) * 128],
                                    kt4[h][:, r, jm0:jm0 + jmw],
                                    qt4[h][:, r, :],
                                    start=True, stop=True, skip_group_check=True)
                        ed2 = epool.tile([128, 2, 512], BF16, tag="e", name=f"ed{qci}_{jmb}")
                        nc.scalar.activation(ed2[0:jmw, :, :], scd2h[0:jmw, :, :],
                                             mybir.ActivationFunctionType.Exp,
                                             scale=EXP_SCALE)
                        last = (jmb == njmb - 1)
                        for r in range(4):
                            for h in (0, 1):
                                off = ((b * 4 + r) * 3 + jmb) * 130 + h * 65
                                nc.tensor.matmul(
                                    ot4[h][:, r, :],
                                    vrr[0:jmw, off:off + 65],
                                    ed2[0:jmw, h, r * 128:(r + 1) * 128],
                                    start=False, stop=(last and r == 3),
                                    skip_group_check=True)

                    # softmax normalize:  O.T = O_aug.T[0:64] / O_aug.T[64],
                    # both heads stacked into one (128, 512) bf16 tile (the
                    # h1 write is partition-shifted 0:64 -> 64:128)
                    rc2 = spool.tile([1, 1024], F32, tag="rc", name=f"rc{qci}")
                    nc.vector.reciprocal(rc2[:, 0:512], ot0[64:65, :])
                    nc.vector.reciprocal(rc2[:, 512:1024], ot1[64:65, :])
                    rb2 = spool.tile([64, 1024], F32, tag="rb", name=f"rb{qci}")
                    nc.gpsimd.partition_broadcast(rb2, rc2)
                    ot2 = opool.tile([128, 512], BF16, tag="ot2", name=f"ot2_{qci}")
                    nc.vector.tensor_mul(ot2[0:64, :], ot0[0:64, :], rb2[:, 0:512])
                    nc.vector.tensor_mul(ot2[64:128, :], ot1[0:64, :], rb2[:, 512:1024])

                    # output projection: single K=128 matmuls per (ib, oc);
                    # PSUM->SBUF copies spread over ACT/DVE/Pool
                    ysb2 = ypool.tile([128, 4, D], BF16, tag="ysb", name=f"ysb{qci}")
                    # Pool/GPSIMD cannot touch PSUM (BIR verifier), so the
                    # Y staging copies alternate between ACT and DVE only.
                    cp_act = lambda dst, src: nc.scalar.copy(dst, src)
                    cp_dve = lambda dst, src: nc.vector.tensor_copy(dst, src)
                    copy_engines = (
                        (cp_act, cp_dve), (cp_dve, cp_act),
                        (cp_act, cp_dve), (cp_dve, cp_act),
                    )
                    for ib in range(4):
                        py2 = psc.tile([128, 1024], F32, tag="sc",
                                       name=f"py{qci}_{ib}")
                        for oc in range(2):
                            nc.tensor.matmul(py2[:, oc * 512:(oc + 1) * 512],
                                             ot2[:, ib * 128:(ib + 1) * 128],
                                             wo_sb[:, oc * 512:(oc + 1) * 512],
                                             start=True, stop=True,
                                             skip_group_check=True)
                        e0, e1 = copy_engines[ib]
                        e0(ysb2[:, ib, 0:512], py2[:, 0:512])
                        e1(ysb2[:, ib, 512:1024], py2[:, 512:1024])
                    row0 = b * S + c * 512
                    yv = y[row0:row0 + 512, :].rearrange("(i p) o -> p i o", p=128)
                    nc.sync.dma_start(yv[:, 0:2, :], ysb2[:, 0:2, :])
                    nc.sync.dma_start(yv[:, 2:4, :], ysb2[:, 2:4, :])

                for cc in range(CPB):
                    # ---------- projections for chunk cc ----------
                    ci = bb * CPB + cc
                    n0 = ci * 512
                    xtb = xpool.tile([128, 8, 512], BF16, tag="xt", name=f"xt{ci}")
                    xt8 = x8pool.tile([128, 8, 512], FP8, tag="x8", name=f"x8{ci}")
                    if ci == 0 or ci == CPB:
                        # per-piece loads so matmuls start on the first tiles
                        for pr in range(4):
                            nc.sync.dma_start(xt8[:, 2 * pr:2 * pr + 2, :],
                                              xT8[:, 2 * pr:2 * pr + 2, n0:n0 + 512])
                        for ft in range(8):
                            nc.sync.dma_start(xtb[:, ft, :], xT[:, ft, n0:n0 + 512])
                        if ci == 0:
                            nc.sync.dma_start(wo_sb, woT)
                            nc.sync.dma_start(w01_sb, w01)
                    else:
                        nc.sync.dma_start(xt8, xT8[:, :, n0:n0 + 512])
                        nc.sync.dma_start(xtb, xT[:, :, n0:n0 + 512])

                    # Q/K via fp8 DoubleRow (two f-tiles per instruction)
                    psq = pp1.tile([128, 512], F32, tag="proj", name=f"psq{ci}")
                    psk = pp1.tile([128, 512], F32, tag="proj", name=f"psk{ci}")
                    for pr in range(4):
                        nc.tensor.matmul(psq, wqk_sb[:, 2 * pr:2 * pr + 2, 0:DH2],
                                         xt8[:, 2 * pr:2 * pr + 2, :],
                                         start=(pr == 0), stop=(pr == 3),
                                         perf_mode=DR)
                    for pr in range(4):
                        nc.tensor.matmul(psk, wqk_sb[:, 2 * pr:2 * pr + 2, DH2:2 * DH2],
                                         xt8[:, 2 * pr:2 * pr + 2, :],
                                         start=(pr == 0), stop=(pr == 3),
                                         perf_mode=DR)
                    qt = qkpool.tile([128, 512], BF16, tag="qt", name=f"qt{ci}")
                    nc.scalar.copy(qt, psq)
                    qts[ci] = qt
                    nc.scalar.copy(kt[:, cc * 512:cc * 512 + 512], psk)

                    psv = pp1.tile([128, 512], F32, tag="proj", name=f"psv{ci}")
                    for ft in range(8):
                        nc.tensor.matmul(psv, wv_sb[:, ft, :], xtb[:, ft, :],
                                         start=(ft == 0), stop=(ft == 7))
                    nc.scalar.copy(vt[:, cc * 512:cc * 512 + 512], psv)

                    # natural V blocks for this chunk
                    for sub in range(4):
                        g = ci * 4 + sub
                        pvt = pp1.tile([128, 128], BF16, tag="proj", name=f"pvt{g}")
                        nc.tensor.transpose(
                            pvt, vt[:, cc * 512 + sub * 128:cc * 512 + (sub + 1) * 128],
                            ident)
                        dst = vhh[:, g * 130:g * 130 + 130].rearrange(
                            "p (two w) -> p two w", two=2)[:, :, 0:64]
                        nc.vector.tensor_copy(
                            dst, pvt.rearrange("p (two w) -> p two w", two=2))
                    # residue-gathered V_r block cc (needed by chunks > cc)
                    if cc < 3:
                        for r in range(4):
                            pvr = pp1.tile([128, 128], BF16, tag="proj",
                                            name=f"pvr{bb}_{r}_{cc}")
                            nc.tensor.transpose(
                                pvr, vt4[:, r, cc * 128:(cc + 1) * 128], ident)
                            off = ((bb * 4 + r) * 3 + cc) * 130
                            dst = vrr[:, off:off + 130].rearrange(
                                "p (two w) -> p two w", two=2)[:, :, 0:64]
                            nc.vector.tensor_copy(
                                dst, pvr.rearrange("p (two w) -> p two w", two=2))

                    do_attn(cc, ci)


_NC_CACHE = None


def _get_nc():
    global _NC_CACHE
    if _NC_CACHE is None:
        nc = bacc.Bacc("TRN2", target_bir_lowering=False, debug=False,
                       num_devices=NCORES)
        with tile.TileContext(nc) as tc:
            build_tile_kernel(tc)
        nc.compile()
        _NC_CACHE = nc
    return _NC_CACHE


def _mask_band():
    """W01[j', t] = 1 if delta = t - T0 - j' is an allowed attention offset."""
    jj = np.arange(128)[:, None]
    tt = np.arange(TW)[None, :]
    delta = tt - T0 - jj
    win = (delta >= 0) & (delta <= WINDOW - 1)
    dil = (delta >= WINDOW + DIL) & (delta % DIL == 0)
    return (win | dil).astype(NPBF16)


def make_in_maps(x, qkv, wo):
    # xT2[p, ft, n] = x[n, ft*128 + p]
    xn = x.reshape(N, 8, 128).transpose(2, 1, 0)
    xT2 = np.ascontiguousarray(xn).astype(NPBF16)
    xT8 = np.ascontiguousarray(xn).astype(NPFP8)
    w01 = _mask_band()
    in_maps = []
    for d in range(NCORES):
        r0 = d * DH2
        wq = qkv[r0:r0 + DH2, :] * np.float32(WSCALE)
        wk = qkv[D + r0:D + r0 + DH2, :] * np.float32(WSCALE)
        wv = qkv[2 * D + r0:2 * D + r0 + DH2, :]
        qk_cat = np.concatenate([wq.T, wk.T], axis=1)  # (D, 2*DH2)
        wqk = np.ascontiguousarray(
            qk_cat.reshape(8, 128, 2 * DH2).transpose(1, 0, 2)).astype(NPFP8)
        wv2 = np.ascontiguousarray(
            wv.T.reshape(8, 128, DH2).transpose(1, 0, 2)).astype(NPBF16)
        in_maps.append({
            "xT": xT2,
            "xT8": xT8,
            "wqk8": wqk,
            "wv16": wv2,
            "woT": np.ascontiguousarray(wo[:, r0:r0 + DH2].T).astype(NPBF16),
            "w01": w01,
        })
    return in_maps


def run(x, qkv, wo, trace=False):
    nc = _get_nc()
    in_maps = make_in_maps(x, qkv, wo)
    try:
        res = run_bass_kernel_spmd(nc, in_maps, core_ids=list(range(NCORES)),
                                   trace=trace)
    except ModuleNotFoundError:
        # NTFF profiling hook unavailable in this environment
        res = run_bass_kernel_spmd(nc, in_maps, core_ids=list(range(NCORES)),
                                   trace=False)
    acc = None
    for r in res.results:
        part = np.asarray(r["y"], dtype=np.float32)
        acc = part if acc is None else acc + part
    out = acc.reshape(B, S, D).astype(np.float32)
    return out, res


def kernel(x, qkv, wo):
    out, _ = run(np.asarray(x, dtype=np.float32),
                 np.asarray(qkv, dtype=np.float32),
                 np.asarray(wo, dtype=np.float32))
    return out


# revision 28
# speedup vs baseline: 1.3481x; 1.3481x over previous
"""Trainium2 Bass kernel for CustomWindowMHA (sparse window+dilated attention).

Problem (hardcoded):
  x: (2, 2048, 1024) f32, qkv: (3072, 1024) f32, wo: (1024, 1024) f32
  H=16 heads, dh=64, window=128, dilation=4.
  out = softmax(mask(QK^T/8)) V @ wo^T          (B, S, D) f32

Sharding: 16 heads / 8 cores = 2 heads per core (head-parallel).  Each core
computes its 2 heads' attention output O_d (n=4096, 128) and the partial
Y_d = O_d @ wo[:, d-slice]^T (4096, 1024).  Host sums the 8 partials.

Per-core layout (activations chained transposed; fp32 PSUM accumulation):
  Q/K projections run as fp8-e4m3 DoubleRow matmuls (x and wqk scaled by 16,
  two f-tiles of the K=1024 contraction packed per instruction => 4x fewer
  PE cycles); the 1/(16*16*8) score descale is folded into the exp scale.
  V projection stays bf16 (its error hits the output directly).
  Per j-block, both heads' scores land in one two-bank (128, 1024) PSUM
  tile so a single scaled Exp covers them; the 0/1 W01 band mask is applied
  multiplicatively on DVE.  AV accumulates per head into (65, 512) PSUM with
  a ones-column carrying the softmax denominator.  A deep-dilated residue-
  compacted stream (all-valid, maskless) shares the same accumulators.
  Normalization: one reciprocal pair -> single gpsimd partition_broadcast ->
  two DVE muls writing both heads into ONE stacked (128, 512) bf16 tile (the
  second write is partition-shifted), so the output projection runs as plain
  K=128 matmuls with no split/DMA shuffle.
  Software pipelining: chunk c's output projection (4x2 K=128 matmuls +
  ACT/DVE PSUM->SBUF staging + y DMA) is issued only after chunk c+1's
  attention, so the recip->broadcast->mul latency chain hides behind real
  PE work instead of stalling the in-order PE queue.
"""

import os
import sys

import numpy as np

for _p in ("/opt/trn_rl_repo",):
    if _p not in sys.path and os.path.isdir(_p):
        sys.path.append(_p)

import ml_dtypes
import concourse.bacc as bacc
import concourse.bass as bass
import concourse.mybir as mybir
import concourse.tile as tile
from concourse.bass_utils import run_bass_kernel_spmd
from concourse.masks import make_identity

F32 = mybir.dt.float32
BF16 = mybir.dt.bfloat16
FP8 = mybir.dt.float8e4
NPBF16 = ml_dtypes.bfloat16
NPFP8 = ml_dtypes.float8_e4m3
DR = mybir.MatmulPerfMode.DoubleRow

B, S, D = 2, 2048, 1024
H, DH = 16, 64
WINDOW, DIL = 128, 4
NCORES = 8
N = B * S          # 4096 rows total
DH2 = 2 * DH       # 128 dims per core (2 heads)
NCH = N // 512     # 8 n-chunks of 512
CPB = S // 512     # 4 i-chunks per batch
JB = S // 128      # 16 j-blocks per batch
T0 = 384           # W01 band offset: delta = t - T0 - j'
TW = T0 + 512 * (CPB - 1) + 512  # 2432 band width
WSCALE = 16.0                    # fp8 Q/K weight scale
EXP_SCALE = 1.0 / (WSCALE * WSCALE * 8.0)  # descale folded into exp


def build_tile_kernel(tc):
    nc = tc.nc

    xT = nc.dram_tensor("xT", [128, 8, N], BF16, kind="ExternalInput").ap()
    xT8 = nc.dram_tensor("xT8", [128, 8, N], FP8, kind="ExternalInput").ap()
    wqk8 = nc.dram_tensor("wqk8", [128, 8, 2 * DH2], FP8, kind="ExternalInput").ap()
    wv16 = nc.dram_tensor("wv16", [128, 8, DH2], BF16, kind="ExternalInput").ap()
    woT = nc.dram_tensor("woT", [DH2, D], BF16, kind="ExternalInput").ap()
    w01 = nc.dram_tensor("w01", [128, TW], BF16, kind="ExternalInput").ap()
    y = nc.dram_tensor("y", [N, D], BF16, kind="ExternalOutput").ap()

    with (
        tc.tile_pool(name="const", bufs=1) as cpool,
        tc.tile_pool(name="xin", bufs=3) as xpool,
        tc.tile_pool(name="x8in", bufs=3) as x8pool,
        tc.tile_pool(name="qk", bufs=8) as qkpool,
        tc.tile_pool(name="vtp", bufs=2) as vtpool,
        tc.tile_pool(name="pers", bufs=1) as perspool,
        tc.tile_pool(name="exp", bufs=4) as epool,
        tc.tile_pool(name="ot2", bufs=4) as opool,
        tc.tile_pool(name="small", bufs=4) as spool,
        tc.tile_pool(name="yst", bufs=2) as ypool,
        tc.tile_pool(name="pp1", bufs=2, space="PSUM") as pp1,
        tc.tile_pool(name="psc", bufs=2, space="PSUM") as psc,
        tc.tile_pool(name="pot", bufs=2, space="PSUM") as pot,
    ):
        # ---- constants / weights in SBUF ----
        wqk_sb = cpool.tile([128, 8, 2 * DH2], FP8, name="wqk_sb")
        wv_sb = cpool.tile([128, 8, DH2], BF16, name="wv_sb")
        wo_sb = cpool.tile([128, D], BF16, name="wo_sb")
        w01_sb = cpool.tile([128, TW], BF16, name="w01_sb")
        ident = cpool.tile([128, 128], BF16, name="ident")
        make_identity(nc, ident)

        # per-head V with ones column, interleaved: per j-block g a 130-col
        # group [V_h0(64) | 1 | V_h1(64) | 1]
        vhh = perspool.tile([128, 2 * JB * 130], BF16, name="vhh")
        nc.vector.memset(
            vhh.rearrange("p (g w) -> p g w", w=65)[:, :, 64:65], 1.0)
        # residue-gathered V for the compact stream: (b, r, jmb<3) 130-groups
        vrr = perspool.tile([128, B * 4 * 3 * 130], BF16, name="vrr")
        nc.vector.memset(
            vrr.rearrange("p (g w) -> p g w", w=65)[:, :, 64:65], 1.0)

        qts = [None] * NCH
        ktb = [None] * B
        vtb = [None] * B

        def attn_accum(b, c, qci, bg):
            # ---------- attention for chunk: fills ot0/ot1.  `bg` is a list
            # of deferred output-projection pieces of the PREVIOUS chunk,
            # interleaved between j-block iterations so their PSUM-staging
            # latency hides behind scores/AV work on the in-order PE queue.
            # pace the deferred pieces evenly across window-jb and deep-jmb
            # iterations so the deep stream also has interleaved PE work
            nwin = 4 * c + 4 - max(0, 4 * c - 1) - 1
            ndeep = (max(128 * (4 * c - 1), 0) // 4 + 127) // 128
            nslots = max(nwin + ndeep, 1)
            npieces = len(bg)
            slot = [0]
            def bg_step():
                slot[0] += 1
                want = (npieces * slot[0] + nslots - 1) // nslots
                while bg and (npieces - len(bg)) < want:
                    bg.pop(0)()
            ot0 = pot.tile([65, 512], F32, tag="ot", name=f"ot0_{qci}")
            ot1 = pot.tile([65, 512], F32, tag="ot", name=f"ot1_{qci}")
            ots = (ot0, ot1)
            ot4 = [o.rearrange("p (i r) -> p r i", r=4) for o in ots]
            jcap = 128 * (4 * c - 1)
            jb_lo = max(0, 4 * c - 1)
            qt4 = [qts[qci][h * 64:(h + 1) * 64, :].rearrange(
                "p (i r) -> p r i", r=4) for h in (0, 1)]
            kt4 = [ktb[b][h * 64:(h + 1) * 64, :].rearrange(
                "p (j r) -> p r j", r=4) for h in (0, 1)]

            # ---- window / mixed stream ----
            for jb in range(jb_lo, 4 * c + 4):
                if jb > jb_lo:
                    bg_step()
                g = b * JB + jb
                im0 = max(0, 128 * jb - 512 * c)
                sc2 = psc.tile([128, 1024], F32, tag="sc",
                               name=f"sc{qci}_{jb}")
                sc2h = sc2.rearrange("p (h i) -> p h i", h=2)
                for h in (0, 1):
                    nc.tensor.matmul(
                        sc2h[:, h, im0:512],
                        ktb[b][h * 64:(h + 1) * 64, jb * 128:(jb + 1) * 128],
                        qts[qci][h * 64:(h + 1) * 64, im0:512],
                        start=True, stop=True, skip_group_check=True)
                e2 = epool.tile([128, 2, 512], BF16, tag="e", name=f"e{qci}_{jb}")
                nc.scalar.activation(e2[:, :, im0:512], sc2h[:, :, im0:512],
                                     mybir.ActivationFunctionType.Exp,
                                     scale=EXP_SCALE)
                em2 = epool.tile([128, 2, 512], BF16, tag="em", name=f"em{qci}_{jb}")
                t0 = T0 + 512 * c - 128 * jb
                wsl = w01_sb[:, t0 + im0:t0 + 512]
                wb = bass.AP(tensor=wsl.tensor, offset=wsl.offset,
                             ap=[list(wsl.ap[0]), [0, 2], list(wsl.ap[1])])
                nc.vector.tensor_mul(em2[:, :, im0:512], e2[:, :, im0:512], wb)
                for h in (0, 1):
                    nc.tensor.matmul(
                        ots[h][:, im0:512],
                        vhh[:, g * 130 + h * 65:g * 130 + h * 65 + 65],
                        em2[:, h, im0:512],
                        start=(jb == jb_lo), stop=False, skip_group_check=True)

            # ---- deep-dilated compact stream (all-valid; maskless) ----
            jmtot = max(jcap, 0) // 4
            njmb = (jmtot + 127) // 128
            for jmb in range(njmb):
                bg_step()
                jm0 = 128 * jmb
                jmw = min(128, jmtot - jm0)
                scd2 = psc.tile([128, 1024], F32, tag="sc",
                                name=f"scd{qci}_{jmb}")
                scd2h = scd2.rearrange("p (h i) -> p h i", h=2)
                for r in range(4):
                    for h in (0, 1):
                        nc.tensor.matmul(
                            scd2h[0:jmw, h, r * 128:(r + 1) * 128],
                            kt4[h][:, r, jm0:jm0 + jmw],
                            qt4[h][:, r, :],
                            start=True, stop=True, skip_group_check=True)
                ed2 = epool.tile([128, 2, 512], BF16, tag="e", name=f"ed{qci}_{jmb}")
                nc.scalar.activation(ed2[0:jmw, :, :], scd2h[0:jmw, :, :],
                                     mybir.ActivationFunctionType.Exp,
                                     scale=EXP_SCALE)
                last = (jmb == njmb - 1)
                for r in range(4):
                    for h in (0, 1):
                        off = ((b * 4 + r) * 3 + jmb) * 130 + h * 65
                        nc.tensor.matmul(
                            ot4[h][:, r, :],
                            vrr[0:jmw, off:off + 65],
                            ed2[0:jmw, h, r * 128:(r + 1) * 128],
                            start=False, stop=(last and r == 3),
                            skip_group_check=True)

            while bg:
                bg_step()
            return (b, c, qci, ot0, ot1)

        def attn_norm(state, split=False):
            # softmax normalize:  O.T = O_aug.T[0:64] / O_aug.T[64],
            # both heads stacked into one (128, 512) bf16 tile (the
            # h1 write is partition-shifted 0:64 -> 64:128).  With
            # split=True (final chunk) the chain runs in two half-width
            # pieces so the first output-projection blocks start sooner.
            b, c, qci, ot0, ot1 = state
            ot2 = opool.tile([128, 512], BF16, tag="ot2", name=f"ot2_{qci}")
            rc2 = spool.tile([1, 1024], F32, tag="rc", name=f"rc{qci}")
            nhalf = 2 if split else 1
            w = 512 // nhalf
            for hf in range(nhalf):
                cs, ce = hf * w, (hf + 1) * w
                # [den_h0(w) | den_h1(w)] contiguous per half
                r0 = hf * 2 * w
                nc.vector.reciprocal(rc2[:, r0:r0 + w], ot0[64:65, cs:ce])
                nc.vector.reciprocal(rc2[:, r0 + w:r0 + 2 * w], ot1[64:65, cs:ce])
                rb2 = spool.tile([64, 2 * w], F32, tag="rb", name=f"rb{qci}_{hf}")
                nc.gpsimd.partition_broadcast(rb2, rc2[:, r0:r0 + 2 * w])
                nc.vector.tensor_mul(ot2[0:64, cs:ce], ot0[0:64, cs:ce],
                                     rb2[:, 0:w])
                nc.vector.tensor_mul(ot2[64:128, cs:ce], ot1[0:64, cs:ce],
                                     rb2[:, w:2 * w])
            return (b, c, qci, ot2)

        def oproj_pieces(state):
            # ---------- deferred output projection + y store, split into 4
            # independently issuable pieces (one per 128-row block) ----------
            b, c, qci, ot2 = state
            cp_act = lambda dst, src: nc.scalar.copy(dst, src)
            cp_dve = lambda dst, src: nc.vector.tensor_copy(dst, src)
            copy_engines = (
                (cp_act, cp_dve), (cp_dve, cp_act),
                (cp_act, cp_dve), (cp_act, cp_act),
            )
            ysb2 = ypool.tile([128, 4, D], BF16, tag="ysb", name=f"ysb{qci}")
            row0 = b * S + c * 512
            yv = y[row0:row0 + 512, :].rearrange("(i p) o -> p i o", p=128)

            def mk(ib):
                def piece():
                    e01 = copy_engines[ib]
                    for oc in range(2):
                        py = pp1.tile([128, 512], F32, tag="proj",
                                      name=f"py{qci}_{ib}_{oc}")
                        nc.tensor.matmul(py,
                                         ot2[:, ib * 128:(ib + 1) * 128],
                                         wo_sb[:, oc * 512:(oc + 1) * 512],
                                         start=True, stop=True,
                                         skip_group_check=True)
                        e01[oc](ysb2[:, ib, oc * 512:(oc + 1) * 512], py)
                    nc.sync.dma_start(yv[:, ib:ib + 1, :], ysb2[:, ib:ib + 1, :])
                return piece
            return [mk(ib) for ib in range(4)]

        pending = None
        for ci in range(NCH):
            bb, cc = divmod(ci, CPB)
            if cc == 0:
                qkt = qkpool.tile([128, 2, S], BF16, tag="qkt", name=f"qkt{bb}",
                                  bufs=2)
                vt = vtpool.tile([128, S], BF16, tag="vt", name=f"vtb{bb}", bufs=2)
                ktb[bb] = qkt[:, 1, :]
                vtb[bb] = vt
                qktb = qkt
                vt4 = vt.rearrange("p (j r) -> p r j", r=4)
            qkt, vt = qktb, vtb[bb]

            # ---------- projections for chunk ----------
            n0 = ci * 512
            xtb = xpool.tile([128, 8, 512], BF16, tag="xt", name=f"xt{ci}")
            xt8 = x8pool.tile([128, 8, 512], FP8, tag="x8", name=f"x8{ci}")
            if ci == 0:
                # interleave weight + x loads so both the DoubleRow Q/K chain
                # and the bf16 V chain start as early as possible
                nc.sync.dma_start(wv_sb, wv16)
                for pr in range(4):
                    nc.sync.dma_start(wqk_sb[:, 2 * pr:2 * pr + 2, :],
                                      wqk8[:, 2 * pr:2 * pr + 2, :])
                    nc.sync.dma_start(xt8[:, 2 * pr:2 * pr + 2, :],
                                      xT8[:, 2 * pr:2 * pr + 2, n0:n0 + 512])
                    nc.sync.dma_start(xtb[:, 2 * pr:2 * pr + 2, :],
                                      xT[:, 2 * pr:2 * pr + 2, n0:n0 + 512])
                nc.sync.dma_start(wo_sb, woT)
                nc.sync.dma_start(w01_sb, w01)
            elif ci == CPB:
                # batch-1 head chunk competes with batch-0 y stores
                for pr in range(4):
                    nc.sync.dma_start(xt8[:, 2 * pr:2 * pr + 2, :],
                                      xT8[:, 2 * pr:2 * pr + 2, n0:n0 + 512])
                for ft in range(8):
                    nc.sync.dma_start(xtb[:, ft, :], xT[:, ft, n0:n0 + 512])
            else:
                nc.sync.dma_start(xt8, xT8[:, :, n0:n0 + 512])
                nc.sync.dma_start(xtb, xT[:, :, n0:n0 + 512])

            # Q/K via fp8 DoubleRow (two f-tiles per instruction), both
            # heads' Q and K into one two-bank PSUM tile -> single ACT copy
            psqk = psc.tile([128, 1024], F32, tag="sc", name=f"psqk{ci}")
            for pr in range(4):
                nc.tensor.matmul(psqk[:, 0:512],
                                 wqk_sb[:, 2 * pr:2 * pr + 2, 0:DH2],
                                 xt8[:, 2 * pr:2 * pr + 2, :],
                                 start=(pr == 0), stop=(pr == 3),
                                 perf_mode=DR, skip_group_check=True)
            for pr in range(4):
                nc.tensor.matmul(psqk[:, 512:1024],
                                 wqk_sb[:, 2 * pr:2 * pr + 2, DH2:2 * DH2],
                                 xt8[:, 2 * pr:2 * pr + 2, :],
                                 start=(pr == 0), stop=(pr == 3),
                                 perf_mode=DR, skip_group_check=True)
            nc.scalar.copy(qkt[:, :, cc * 512:cc * 512 + 512],
                           psqk.rearrange("p (g i) -> p g i", g=2))
            qts[ci] = qkt[:, 0, cc * 512:cc * 512 + 512]

            psv = pp1.tile([128, 512], F32, tag="proj", name=f"psv{ci}")
            for ft in range(8):
                nc.tensor.matmul(psv, wv_sb[:, ft, :], xtb[:, ft, :],
                                 start=(ft == 0), stop=(ft == 7))
            nc.vector.tensor_copy(vt[:, cc * 512:cc * 512 + 512], psv)

            # natural V blocks for this chunk
            for sub in range(4):
                g = ci * 4 + sub
                pvt = pp1.tile([128, 128], BF16, tag="proj", name=f"pvt{g}")
                nc.tensor.transpose(
                    pvt, vt[:, cc * 512 + sub * 128:cc * 512 + (sub + 1) * 128],
                    ident)
                dst = vhh[:, g * 130:g * 130 + 130].rearrange(
                    "p (two w) -> p two w", two=2)[:, :, 0:64]
                nc.vector.tensor_copy(
                    dst, pvt.rearrange("p (two w) -> p two w", two=2))
            # residue-gathered V_r block cc (needed by chunks > cc)
            if cc < 3:
                for r in range(4):
                    pvr = pp1.tile([128, 128], BF16, tag="proj",
                                    name=f"pvr{bb}_{r}_{cc}")
                    nc.tensor.transpose(
                        pvr, vt4[:, r, cc * 128:(cc + 1) * 128], ident)
                    off = ((bb * 4 + r) * 3 + cc) * 130
                    dst = vrr[:, off:off + 130].rearrange(
                        "p (two w) -> p two w", two=2)[:, :, 0:64]
                    nc.vector.tensor_copy(
                        dst, pvr.rearrange("p (two w) -> p two w", two=2))

            acc_state = attn_accum(bb, cc, ci, pending or [])
            pending = oproj_pieces(attn_norm(acc_state, split=(ci == NCH - 1)))
        for piece in pending:
            piece()


_NC_CACHE = None


def _get_nc():
    global _NC_CACHE
    if _NC_CACHE is None:
        nc = bacc.Bacc("TRN2", target_bir_lowering=False, debug=False,
                       num_devices=NCORES)
        with tile.TileContext(nc) as tc:
            build_tile_kernel(tc)
        nc.compile()
        _NC_CACHE = nc
    return _NC_CACHE


def _mask_band():
    """W01[j', t] = 1 if delta = t - T0 - j' is an allowed attention offset."""
    jj = np.arange(128)[:, None]
    tt = np.arange(TW)[None, :]
    delta = tt - T0 - jj
    win = (delta >= 0) & (delta <= WINDOW - 1)
    dil = (delta >= WINDOW + DIL) & (delta % DIL == 0)
    return (win | dil).astype(NPBF16)


def make_in_maps(x, qkv, wo):
    # xT2[p, ft, n] = x[n, ft*128 + p]
    xn = x.reshape(N, 8, 128).transpose(2, 1, 0)
    xT2 = np.ascontiguousarray(xn).astype(NPBF16)
    xT8v = np.ascontiguousarray(xn).astype(NPFP8)
    w01 = _mask_band()
    in_maps = []
    for d in range(NCORES):
        r0 = d * DH2
        wq = qkv[r0:r0 + DH2, :] * np.float32(WSCALE)
        wk = qkv[D + r0:D + r0 + DH2, :] * np.float32(WSCALE)
        wv = qkv[2 * D + r0:2 * D + r0 + DH2, :]
        qk_cat = np.concatenate([wq.T, wk.T], axis=1)  # (D, 2*DH2)
        wqk = np.ascontiguousarray(
            qk_cat.reshape(8, 128, 2 * DH2).transpose(1, 0, 2)).astype(NPFP8)
        wv2 = np.ascontiguousarray(
            wv.T.reshape(8, 128, DH2).transpose(1, 0, 2)).astype(NPBF16)
        in_maps.append({
            "xT": xT2,
            "xT8": xT8v,
            "wqk8": wqk,
            "wv16": wv2,
            "woT": np.ascontiguousarray(wo[:, r0:r0 + DH2].T).astype(NPBF16),
            "w01": w01,
        })
    return in_maps


def run(x, qkv, wo, trace=False):
    nc = _get_nc()
    in_maps = make_in_maps(x, qkv, wo)
    try:
        res = run_bass_kernel_spmd(nc, in_maps, core_ids=list(range(NCORES)),
                                   trace=trace)
    except ModuleNotFoundError:
        # NTFF profiling hook unavailable in this environment
        res = run_bass_kernel_spmd(nc, in_maps, core_ids=list(range(NCORES)),
                                   trace=False)
    acc = None
    for r in res.results:
        part = np.asarray(r["y"], dtype=np.float32)
        acc = part if acc is None else acc + part
    out = acc.reshape(B, S, D).astype(np.float32)
    return out, res


def kernel(x, qkv, wo):
    out, _ = run(np.asarray(x, dtype=np.float32),
                 np.asarray(qkv, dtype=np.float32),
                 np.asarray(wo, dtype=np.float32))
    return out
